# revision 29
# baseline (speedup 1.0000x reference)
"""Trainium2 Bass kernel for GQA attention block (B=2, S=2048, H=2048, NH=32, NKV=8, HD=64).

Sharding: 8 cores = data-parallel over batch (2) x tensor-parallel over heads (4).
Each core computes the qkv projection for its 8 q-heads / 2 kv-heads, RoPE,
causal GQA attention, and a partial o-projection (its 512 rows of w_o). The
host sums the 4 partial outputs per batch.

Device-side design:
  - x is passed pre-transposed (xT [H, S]) so the hidden dim lives on SBUF
    partitions for the qkv matmuls, producing qkv^T [feat, seq] directly.
  - Interleaved RoPE becomes contiguous half-swaps via a host-side column
    permutation of w_qkv (even dims first, odd dims second).
  - Scores are computed transposed (k on partitions, q free); the softmax
    denominator comes from a ones-row appended to v in the probs@v matmul:
    no probs transpose, no reduction pass.
  - exp() skips max-subtraction (scores ~N(0,1), mathematically identical);
    the 1/sqrt(HD) scale rides the ACT activation's free `scale`.
  - All matmul operands are float16 (10-bit mantissa = TF32-grade accuracy,
    1 cycle/row at any N, overlapped fast weight loads). PSUM accumulation
    stays fp32; score PSUM tiles are fp16 (1024 cols/bank) so one score
    matmul chunk + one exp instruction cover a whole key block.
  - Each 512-wide PV accumulator owns a full PSUM bank (start=True clears
    has_written for the whole bank, so groups must never share one).
"""

import sys

if "/opt/trn_rl_repo" not in sys.path:
    sys.path.insert(0, "/opt/trn_rl_repo")

import numpy as np

import concourse.bass as bass
import concourse.mybir as mybir
import concourse.tile as tile
from concourse import bacc
from concourse.bass_utils import run_bass_kernel_spmd

P = 128
S = 2048
H = 2048
NH = 32
NKV = 8
HD = 64
GROUPS = NH // NKV  # 4
NHL = 8   # local q heads per core
NKVL = 2  # local kv heads per core
FQ = NHL * HD   # 512
F = FQ + 2 * NKVL * HD  # 768
NKB = S // P    # 16 key blocks
ROPE_BASE = 10000.0

F32 = mybir.dt.float32
F16 = mybir.dt.float16


def build_bass():
    nc = bacc.Bacc("TRN2", num_devices=8)

    xT = nc.declare_dram_parameter("xT", [H, S], F16, isOutput=False)
    wqkv = nc.declare_dram_parameter("wqkv", [H, F], F16, isOutput=False)
    wo = nc.declare_dram_parameter("wo", [FQ, H], F16, isOutput=False)
    cosx = nc.declare_dram_parameter("cosx", [P, S], F32, isOutput=False)
    sinx = nc.declare_dram_parameter("sinx", [P, S], F32, isOutput=False)
    tri = nc.declare_dram_parameter("tri", [P, P], F16, isOutput=False)
    idn = nc.declare_dram_parameter("idn", [P, 64], F16, isOutput=False)
    ones = nc.declare_dram_parameter("ones", [P, 1], F16, isOutput=False)
    out = nc.declare_dram_parameter("out", [S, H], F32, isOutput=True)

    with tile.TileContext(nc) as tc:
        with (
            tc.tile_pool(name="const", bufs=1) as const,
            tc.tile_pool(name="wq", bufs=1) as wqp,
            tc.tile_pool(name="qkvT", bufs=1) as qkvp,
            tc.tile_pool(name="vsb", bufs=1) as vsbp,
            tc.tile_pool(name="attnT", bufs=1) as attp,
            tc.tile_pool(name="wop", bufs=1) as wop,
        ):
            cos_sb = const.tile([P, S], F32)
            sin_sb = const.tile([P, S], F32)
            tri_sb = const.tile([P, P], F16)
            idn_sb = const.tile([P, 64], F16)
            nc.sync.dma_start(out=cos_sb, in_=cosx.ap())
            nc.sync.dma_start(out=sin_sb, in_=sinx.ap())
            nc.sync.dma_start(out=tri_sb, in_=tri.ap())
            nc.sync.dma_start(out=idn_sb, in_=idn.ap())

            # weights resident: wq_all [128, h-chunk, feat], wo_all [128, c, H]
            wq_all = wqp.tile([P, H // P, F], F16)
            nc.sync.dma_start(
                out=wq_all, in_=wqkv.ap().rearrange("(ho p) f -> p ho f", p=P)
            )
            wo_all = wop.tile([P, 4, H], F16)
            nc.sync.dma_start(
                out=wo_all, in_=wo.ap().rearrange("(c p) n -> p c n", p=P)
            )

            # qkv^T: 4 q chunks (2 heads each); k replicated per kv head at
            # both 64-partition offsets (matmul operands must share their
            # base partition).
            qT_sb = [qkvp.tile([P, S], F16, tag=f"qT{c}", name=f"qT{c}") for c in range(4)]
            kT_rep = [qkvp.tile([P, S], F16, tag=f"kT{h}", name=f"kT{h}") for h in range(NKVL)]
            # v in [seq, hd] layout per kv head and key block, + ones column
            v_sb = [
                [vsbp.tile([P, HD + 1], F16, tag=f"v{hv}_{kb}", name=f"v{hv}_{kb}") for kb in range(NKB)]
                for hv in range(NKVL)
            ]
            attnT_sb = [attp.tile([P, S], F16, tag=f"at{c}", name=f"at{c}") for c in range(4)]

            # ------- PE warmup: ~4.5us of dummy matmuls so the HAM clock
            # gate ramps to 2.4 GHz before the real work arrives.
            with tc.tile_pool(name="wup", bufs=1, space="PSUM") as wupp:
                wup = wupp.tile([P, P], F32)
                for _ in range(40):
                    nc.tensor.matmul(wup, lhsT=tri_sb, rhs=tri_sb, start=True, stop=True)

            # ---------------- Phase 1: qkv^T = wqkv^T @ x^T, RoPE, v transpose
            SCH = 512  # seq chunk width for qkv matmuls
            with (
                tc.tile_pool(name="xw", bufs=4) as xw,
                tc.tile_pool(name="p1ps", bufs=1, space="PSUM") as p1ps,
                tc.tile_pool(name="p1vt", bufs=2, space="PSUM") as p1vt,
                tc.tile_pool(name="rtmp", bufs=3) as rtmp,
                tc.tile_pool(name="vtt", bufs=2) as vtt,
            ):
                for s in range(S // SCH):
                    ssl = slice(s * SCH, (s + 1) * SCH)
                    psums = [
                        p1ps.tile([P, SCH], F32, tag=f"qkv{f}", name=f"qkv{f}") for f in range(6)
                    ]
                    for h in range(H // P):
                        xt = xw.tile([P, SCH], F16, tag="xt")
                        nc.sync.dma_start(out=xt, in_=xT[h * P:(h + 1) * P, ssl])
                        for f in range(6):
                            nc.tensor.matmul(
                                psums[f],
                                lhsT=wq_all[:, h, f * P:(f + 1) * P],
                                rhs=xt,
                                start=(h == 0),
                                stop=(h == H // P - 1),
                            )
                    # RoPE for q (f=0..3) and k (f=4); deinterleaved halves:
                    #   out_even = t_even*cos - t_odd*sin
                    #   out_odd  = t_even*sin + t_odd*cos
                    for f in range(5):
                        t = psums[f]
                        for hh in range(2):       # 64-row head within the chunk
                            for par in range(2):  # 0 = even half, 1 = odd half
                                p0 = hh * 64 + par * 32
                                q0 = hh * 64 + (1 - par) * 32
                                ta = rtmp.tile([32, SCH], F32, tag="ra", name="ra")
                                tb = rtmp.tile([32, SCH], F32, tag="rb", name="rb")
                                nc.vector.tensor_mul(
                                    ta, t[p0:p0 + 32, :], cos_sb[p0:p0 + 32, ssl]
                                )
                                nc.vector.tensor_mul(
                                    tb, t[q0:q0 + 32, :], sin_sb[q0:q0 + 32, ssl]
                                )
                                op = (
                                    mybir.AluOpType.subtract
                                    if par == 0
                                    else mybir.AluOpType.add
                                )
                                if f < 4:
                                    nc.vector.tensor_tensor(
                                        qT_sb[f][p0:p0 + 32, ssl], ta, tb, op
                                    )
                                else:
                                    # k: hh is the kv head; write both offsets
                                    nc.vector.tensor_tensor(
                                        kT_rep[hh][par * 32:par * 32 + 32, ssl],
                                        ta, tb, op,
                                    )
                                    nc.vector.tensor_tensor(
                                        kT_rep[hh][64 + par * 32:64 + par * 32 + 32, ssl],
                                        ta, tb, op,
                                    )
                    # v: psums[5] is v^T [2 kv heads x 64, SCH] -> [seq, hd]
                    vt = vtt.tile([P, SCH], F16, tag="vt")
                    nc.vector.tensor_copy(out=vt, in_=psums[5])
                    for hv in range(NKVL):
                        for j in range(SCH // P):
                            kb = (s * SCH + j * P) // P
                            pvtr = p1vt.tile([P, HD], F16, tag="vtp", name="vtp")
                            nc.tensor.transpose(
                                pvtr,
                                vt[hv * HD:(hv + 1) * HD, j * P:(j + 1) * P],
                                idn_sb[hv * HD:(hv + 1) * HD, :],
                            )
                            nc.vector.tensor_copy(out=v_sb[hv][kb][:, 0:HD], in_=pvtr)
                            nc.sync.dma_start(
                                out=v_sb[hv][kb][:, HD:HD + 1], in_=ones.ap()
                            )

            # ---------------- Phase 2: attention, one pass per head.
            # Score PSUM is fp16 (1024 cols/bank): one matmul chunk per bank,
            # one exp per (head, key block). PV: 4 fp32 accumulators of 512
            # columns, one full bank each.
            with (
                tc.tile_pool(name="probs", bufs=3) as prp,
                tc.tile_pool(name="p2sc", bufs=2, space="PSUM") as p2sc,
                tc.tile_pool(name="p2pv", bufs=1, space="PSUM") as p2pv,
                tc.tile_pool(name="dvt", bufs=2) as dvt,
            ):
                for hl in range(NHL):
                    qc, qoff = hl // 2, (hl % 2) * HD
                    hv = hl // GROUPS
                    kTh = kT_rep[hv][qoff:qoff + HD, :]
                    qTh = qT_sb[qc][qoff:qoff + HD, :]
                    pvt = [
                        p2pv.tile([HD + 1, 512], F32, tag=f"pv{g}", name=f"pv{g}")
                        for g in range(4)
                    ]
                    for kb in range(NKB):
                        q0 = kb * P
                        pt = prp.tile([P, S], F16, tag="pt")
                        # score chunks of 1024 (2 fp32 PSUM banks, 2 matmuls),
                        # one exp instruction per chunk
                        ch0 = q0
                        while ch0 < S:
                            cw = min(1024 - ch0 % 1024, S - ch0)
                            sc = p2sc.tile([P, 1024], F32, tag="sc")
                            mm0 = 0
                            while mm0 < cw:
                                mw = min(512, cw - mm0)
                                nc.tensor.matmul(
                                    sc[:, mm0:mm0 + mw],
                                    lhsT=kTh[:, q0:q0 + P],
                                    rhs=qTh[:, ch0 + mm0:ch0 + mm0 + mw],
                                    start=True,
                                    stop=True,
                                )
                                mm0 += mw
                            # exp(score/8): the 1/sqrt(HD) scale rides the ACT
                            nc.scalar.activation(
                                out=pt[:, ch0:ch0 + cw],
                                in_=sc[:, 0:cw],
                                func=mybir.ActivationFunctionType.Exp,
                                scale=0.125,
                            )
                            ch0 += cw
                        # causal mask on the diagonal block (multiply after exp)
                        nc.vector.tensor_mul(
                            pt[:, q0:q0 + P], pt[:, q0:q0 + P], tri_sb
                        )
                        # PV accumulation: probs^T @ [v | 1]
                        for g in range(4):
                            glo, ghi = g * 512, (g + 1) * 512
                            if ghi <= q0:
                                continue  # fully masked for this kb
                            lo = max(glo, q0)
                            nc.tensor.matmul(
                                pvt[g][:, lo - glo:512],
                                lhsT=v_sb[hv][kb][:, 0:HD + 1],
                                rhs=pt[:, lo:ghi],
                                start=(kb == 0),
                                stop=(kb == 4 * g + 3),
                            )
                    # normalize: attnT[d, q] = pv[d, q] / pv[64, q]
                    for g in range(4):
                        den = dvt.tile([1, 512], F32, tag="den", name="den")
                        # custom-DVE ops read from physical partition 0 —
                        # copy the denominator row down first.
                        nc.vector.tensor_copy(out=den, in_=pvt[g][HD:HD + 1, :])
                        rc = dvt.tile([1, 512], F32, tag="rc", name="rc")
                        # denominators are sums of exp() > 0 — approx_fast
                        # edge cases (0/denorm/inf) cannot occur.
                        nc.vector.reciprocal_approx_fast(out=rc, in_=den)
                        rcb = dvt.tile([HD, 512], F32, tag="rcb", name="rcb")
                        nc.gpsimd.partition_broadcast(rcb, rc, channels=HD)
                        osl = slice(g * 512, (g + 1) * 512)
                        nc.vector.tensor_mul(
                            attnT_sb[qc][qoff:qoff + HD, osl], pvt[g][0:HD, :], rcb
                        )

            # ---------------- Phase 3: partial o = attn @ w_o_part
            OCH = 512
            with (
                tc.tile_pool(name="p3ps", bufs=2, space="PSUM") as p3ps,
                tc.tile_pool(name="osb", bufs=3) as osb,
            ):
                for qb in range(S // P):
                    for nch in range(H // OCH):
                        po = p3ps.tile([P, OCH], F32, tag="po")
                        for c in range(4):
                            nc.tensor.matmul(
                                po,
                                lhsT=attnT_sb[c][:, qb * P:(qb + 1) * P],
                                rhs=wo_all[:, c, nch * OCH:(nch + 1) * OCH],
                                start=(c == 0),
                                stop=(c == 3),
                            )
                        ot = osb.tile([P, OCH], F32, tag="ot")
                        nc.any.tensor_copy(out=ot, in_=po)
                        nc.sync.dma_start(
                            out=out[qb * P:(qb + 1) * P, nch * OCH:(nch + 1) * OCH],
                            in_=ot,
                        )

    nc.compile()
    return nc


def _host_tables():
    inv = (1.0 / ROPE_BASE ** (np.arange(0, HD, 2) / HD)).astype(np.float64)  # [32]
    ang = np.arange(S, dtype=np.float64)[:, None] * inv[None, :]  # [S, 32]
    cos32 = np.cos(ang).T.astype(np.float32)  # [32, S]
    sin32 = np.sin(ang).T.astype(np.float32)
    cosx = np.tile(cos32, (4, 1))  # [128, S]
    sinx = np.tile(sin32, (4, 1))
    tri = (np.arange(P)[None, :] >= np.arange(P)[:, None]).astype(np.float16)
    idn = np.concatenate([np.eye(64, dtype=np.float16)] * 2, axis=0)  # [128, 64]
    return cosx, sinx, tri, idn


_PERM = np.concatenate([np.arange(0, HD, 2), np.arange(1, HD, 2)])  # deinterleave


def make_in_maps(x, w_qkv, w_o):
    """Build the 8 per-core input maps from full inputs."""
    cosx, sinx, tri, idn = _host_tables()
    in_maps = []
    for c in range(8):
        b, g = c // 4, c % 4
        xTc = np.ascontiguousarray(x[b].T).astype(np.float16)
        cols = []
        for hq in range(NHL * g, NHL * (g + 1)):
            cols.append(hq * HD + _PERM)
        qcols = np.concatenate(cols)
        cols = []
        for kv in range(NKVL * g, NKVL * (g + 1)):
            cols.append(H + kv * HD + _PERM)
        kcols = np.concatenate(cols)
        cols = []
        for kv in range(NKVL * g, NKVL * (g + 1)):
            cols.append(H + NKV * HD + kv * HD + np.arange(HD))
        vcols = np.concatenate(cols)
        wc = np.concatenate(
            [w_qkv[:, qcols], w_qkv[:, kcols], w_qkv[:, vcols]], axis=1
        ).astype(np.float16)
        woc = w_o[FQ * g:FQ * (g + 1), :].astype(np.float16)
        in_maps.append(
            {
                "xT": xTc,
                "wqkv": wc,
                "wo": woc,
                "cosx": cosx,
                "sinx": sinx,
                "tri": tri,
                "idn": idn,
                "ones": np.ones((P, 1), dtype=np.float16),
            }
        )
    return in_maps


_NC = None


def get_nc():
    global _NC
    if _NC is None:
        _NC = build_bass()
    return _NC


def kernel(x, mask, w_qkv, w_o):
    x = np.asarray(x)
    w_qkv = np.asarray(w_qkv)
    w_o = np.asarray(w_o)
    nc = get_nc()
    in_maps = make_in_maps(x, w_qkv, w_o)
    res = run_bass_kernel_spmd(nc, in_maps, core_ids=list(range(8)))
    out = np.zeros((2, S, H), dtype=np.float32)
    for c in range(8):
        out[c // 4] += res.results[c]["out"]
    return out


# revision 32
# speedup vs baseline: 1.2586x; 1.2586x over previous
"""Trainium2 Bass kernel for GQA attention block (B=2, S=2048, H=2048, NH=32, NKV=8, HD=64).

Sharding: 8 cores = data-parallel over batch (2) x tensor-parallel over heads (4).
Each core computes the qkv projection for its 8 q-heads / 2 kv-heads, RoPE,
causal GQA attention, and a partial o-projection (its 512 rows of w_o). The
host sums the 4 partial outputs per batch.

Device-side design:
  - x is passed pre-transposed (xT [H, S]) so the hidden dim lives on SBUF
    partitions for the qkv matmuls, producing qkv^T [feat, seq] directly.
  - Interleaved RoPE becomes contiguous half-swaps via a host-side column
    permutation of w_qkv (even dims first, odd dims second).
  - Scores are computed transposed (k on partitions, q free); the softmax
    denominator comes from a ones-row appended to v in the probs@v matmul:
    no probs transpose, no reduction pass.
  - exp() skips max-subtraction (scores ~N(0,1), mathematically identical);
    the 1/sqrt(HD) scale rides the ACT activation's free `scale`.
  - All matmul operands are float16 (10-bit mantissa = TF32-grade accuracy,
    1 cycle/row at any N, overlapped fast weight loads). PSUM accumulation
    stays fp32; score PSUM tiles are fp16 (1024 cols/bank) so one score
    matmul chunk + one exp instruction cover a whole key block.
  - Each 512-wide PV accumulator owns a full PSUM bank (start=True clears
    has_written for the whole bank, so groups must never share one).
"""

import sys

if "/opt/trn_rl_repo" not in sys.path:
    sys.path.insert(0, "/opt/trn_rl_repo")

import numpy as np

import concourse.bass as bass
import concourse.mybir as mybir
import concourse.tile as tile
from concourse import bacc
from concourse.bass_utils import run_bass_kernel_spmd

P = 128
S = 2048
H = 2048
NH = 32
NKV = 8
HD = 64
GROUPS = NH // NKV  # 4
NHL = 8   # local q heads per core
NKVL = 2  # local kv heads per core
FQ = NHL * HD   # 512
F = FQ + 2 * NKVL * HD  # 768
NKB = S // P    # 16 key blocks
ROPE_BASE = 10000.0

F32 = mybir.dt.float32
F16 = mybir.dt.float16


def build_bass():
    nc = bacc.Bacc("TRN2", num_devices=8)

    xT = nc.declare_dram_parameter("xT", [H, S], F16, isOutput=False)
    wqkv = nc.declare_dram_parameter("wqkv", [H, F], F16, isOutput=False)
    wo = nc.declare_dram_parameter("wo", [FQ, H], F16, isOutput=False)
    cosx = nc.declare_dram_parameter("cosx", [P, S], F32, isOutput=False)
    sinx = nc.declare_dram_parameter("sinx", [P, S], F32, isOutput=False)
    tri = nc.declare_dram_parameter("tri", [P, P], F16, isOutput=False)
    idn = nc.declare_dram_parameter("idn", [P, 64], F16, isOutput=False)
    ones = nc.declare_dram_parameter("ones", [P, 1], F16, isOutput=False)
    out = nc.declare_dram_parameter("out", [S, H], F32, isOutput=True)

    with tile.TileContext(nc) as tc:
        with (
            tc.tile_pool(name="const", bufs=1) as const,
            tc.tile_pool(name="wq", bufs=1) as wqp,
            tc.tile_pool(name="qkvT", bufs=1) as qkvp,
            tc.tile_pool(name="vsb", bufs=1) as vsbp,
            tc.tile_pool(name="attnT", bufs=1) as attp,
            tc.tile_pool(name="wop", bufs=1) as wop,
        ):
            cos_sb = const.tile([P, S], F32)
            sin_sb = const.tile([P, S], F32)
            tri_sb = const.tile([P, P], F16)
            idn_sb = const.tile([P, 64], F16)
            nc.sync.dma_start(out=cos_sb, in_=cosx.ap())
            nc.sync.dma_start(out=sin_sb, in_=sinx.ap())
            nc.sync.dma_start(out=tri_sb, in_=tri.ap())
            nc.sync.dma_start(out=idn_sb, in_=idn.ap())

            # weights resident: wq_all [128, h-chunk, feat], wo_all [128, c, H]
            wq_all = wqp.tile([P, H // P, F], F16)
            nc.sync.dma_start(
                out=wq_all, in_=wqkv.ap().rearrange("(ho p) f -> p ho f", p=P)
            )
            wo_all = wop.tile([P, 4, H], F16)
            nc.sync.dma_start(
                out=wo_all, in_=wo.ap().rearrange("(c p) n -> p c n", p=P)
            )

            # qkv^T: 4 q chunks (2 heads each); k replicated per kv head at
            # both 64-partition offsets (matmul operands must share their
            # base partition).
            qT_sb = [qkvp.tile([P, S], F16, tag=f"qT{c}", name=f"qT{c}") for c in range(4)]
            kT_rep = [qkvp.tile([P, S], F16, tag=f"kT{h}", name=f"kT{h}") for h in range(NKVL)]
            # v in [seq, hd] layout per kv head and key block, + ones column
            v_sb = [
                [vsbp.tile([P, HD + 1], F16, tag=f"v{hv}_{kb}", name=f"v{hv}_{kb}") for kb in range(NKB)]
                for hv in range(NKVL)
            ]
            attnT_sb = [attp.tile([P, S], F16, tag=f"at{c}", name=f"at{c}") for c in range(4)]

            # ------- PE warmup: ~4.5us of dummy matmuls so the HAM clock
            # gate ramps to 2.4 GHz before the real work arrives.
            with tc.tile_pool(name="wup", bufs=1, space="PSUM") as wupp:
                wup = wupp.tile([P, P], F32)
                for _ in range(40):
                    nc.tensor.matmul(wup, lhsT=tri_sb, rhs=tri_sb, start=True, stop=True)

            # ---------------- Phase 1: qkv^T = wqkv^T @ x^T, RoPE, v transpose
            SCH = 512  # seq chunk width for qkv matmuls
            with (
                tc.tile_pool(name="xw", bufs=4) as xw,
                tc.tile_pool(name="p1ps", bufs=1, space="PSUM") as p1ps,
                tc.tile_pool(name="p1vt", bufs=2, space="PSUM") as p1vt,
                tc.tile_pool(name="rtmp", bufs=3) as rtmp,
                tc.tile_pool(name="vtt", bufs=2) as vtt,
            ):
                for s in range(S // SCH):
                    ssl = slice(s * SCH, (s + 1) * SCH)
                    psums = [
                        p1ps.tile([P, SCH], F32, tag=f"qkv{f}", name=f"qkv{f}") for f in range(6)
                    ]
                    for h in range(H // P):
                        xt = xw.tile([P, SCH], F16, tag="xt")
                        nc.sync.dma_start(out=xt, in_=xT[h * P:(h + 1) * P, ssl])
                        for f in range(6):
                            nc.tensor.matmul(
                                psums[f],
                                lhsT=wq_all[:, h, f * P:(f + 1) * P],
                                rhs=xt,
                                start=(h == 0),
                                stop=(h == H // P - 1),
                            )
                    # RoPE for q (f=0..3) and k (f=4), full-width DVE ops:
                    #   out = t * cos + swap32(t) * sin_signed
                    # where swap32 exchanges adjacent 32-row blocks (even/odd
                    # deinterleaved halves) and sin_signed carries -sin on the
                    # even halves, +sin on the odd. The swapped copy is made
                    # by the otherwise-idle ACT engine (it can read PSUM).
                    for f in range(5):
                        t = psums[f]
                        tsw = rtmp.tile([P, SCH], F32, tag="tsw", name="tsw")
                        for b in range(4):
                            src = (b ^ 1) * 32
                            nc.scalar.copy(
                                out=tsw[b * 32:(b + 1) * 32, :],
                                in_=t[src:src + 32, :],
                            )
                        ta = rtmp.tile([P, SCH], F32, tag="ta", name="ta")
                        nc.vector.tensor_mul(ta, t, cos_sb[:, ssl])
                        nc.vector.tensor_mul(tsw, tsw, sin_sb[:, ssl])
                        if f < 4:
                            nc.vector.tensor_add(qT_sb[f][:, ssl], ta, tsw)
                        else:
                            # k chunk rows: 0:64 = kv0, 64:128 = kv1; write
                            # each kv head at both 64-partition offsets
                            for hh in range(2):
                                si = slice(hh * 64, hh * 64 + 64)
                                nc.vector.tensor_add(
                                    kT_rep[hh][0:64, ssl], ta[si, :], tsw[si, :]
                                )
                                nc.vector.tensor_add(
                                    kT_rep[hh][64:128, ssl], ta[si, :], tsw[si, :]
                                )
                    # v: psums[5] is v^T [2 kv heads x 64, SCH] -> [seq, hd]
                    vt = vtt.tile([P, SCH], F16, tag="vt")
                    nc.vector.tensor_copy(out=vt, in_=psums[5])
                    for hv in range(NKVL):
                        for j in range(SCH // P):
                            kb = (s * SCH + j * P) // P
                            pvtr = p1vt.tile([P, HD], F16, tag="vtp", name="vtp")
                            nc.tensor.transpose(
                                pvtr,
                                vt[hv * HD:(hv + 1) * HD, j * P:(j + 1) * P],
                                idn_sb[hv * HD:(hv + 1) * HD, :],
                            )
                            nc.vector.tensor_copy(out=v_sb[hv][kb][:, 0:HD], in_=pvtr)
                            nc.sync.dma_start(
                                out=v_sb[hv][kb][:, HD:HD + 1], in_=ones.ap()
                            )

            # ---------------- Phase 2: attention, one pass per head.
            # Score PSUM is fp16 (1024 cols/bank): one matmul chunk per bank,
            # one exp per (head, key block). PV: 4 fp32 accumulators of 512
            # columns, one full bank each.
            with (
                tc.tile_pool(name="probs", bufs=3) as prp,
                tc.tile_pool(name="p2sc", bufs=2, space="PSUM") as p2sc,
                tc.tile_pool(name="p2pv", bufs=1, space="PSUM") as p2pv,
                tc.tile_pool(name="dvt", bufs=2) as dvt,
            ):
                for hl in range(NHL):
                    qc, qoff = hl // 2, (hl % 2) * HD
                    hv = hl // GROUPS
                    kTh = kT_rep[hv][qoff:qoff + HD, :]
                    qTh = qT_sb[qc][qoff:qoff + HD, :]
                    pvt = [
                        p2pv.tile([HD + 1, 512], F32, tag=f"pv{g}", name=f"pv{g}")
                        for g in range(4)
                    ]
                    # software-pipelined job list: one job per 1024-col score
                    # chunk. The QK matmuls for job j+1 are emitted BEFORE
                    # exp/PV of job j so the in-order PE stream never sits
                    # behind a PV that is waiting on the ACT exp.
                    jobs = []
                    for kb in range(NKB):
                        ch0 = kb * P
                        while ch0 < S:
                            cw = min(1024 - ch0 % 1024, S - ch0)
                            jobs.append((kb, ch0, cw))
                            ch0 += cw

                    def emit_qk(job):
                        kb, ch0, cw = job
                        q0 = kb * P
                        sc = p2sc.tile([P, 1024], F32, tag="sc")
                        mm0 = 0
                        while mm0 < cw:
                            mw = min(512, cw - mm0)
                            nc.tensor.matmul(
                                sc[:, mm0:mm0 + mw],
                                lhsT=kTh[:, q0:q0 + P],
                                rhs=qTh[:, ch0 + mm0:ch0 + mm0 + mw],
                                start=True,
                                stop=True,
                            )
                            mm0 += mw
                        return sc

                    sc_pending = emit_qk(jobs[0])
                    pt = None
                    for idx, job in enumerate(jobs):
                        kb, ch0, cw = job
                        q0 = kb * P
                        sc = sc_pending
                        if idx + 1 < len(jobs):
                            sc_pending = emit_qk(jobs[idx + 1])
                        if ch0 == q0:  # first chunk of this key block
                            pt = prp.tile([P, S], F16, tag="pt", name="pt")
                        # exp(score/8): the 1/sqrt(HD) scale rides the ACT op
                        nc.scalar.activation(
                            out=pt[:, ch0:ch0 + cw],
                            in_=sc[:, 0:cw],
                            func=mybir.ActivationFunctionType.Exp,
                            scale=0.125,
                        )
                        if ch0 == q0:
                            # causal mask on the diagonal block (mult after exp)
                            nc.vector.tensor_mul(
                                pt[:, q0:q0 + P], pt[:, q0:q0 + P], tri_sb
                            )
                        if ch0 + cw == S:  # last chunk: PV for this key block
                            for g in range(4):
                                glo, ghi = g * 512, (g + 1) * 512
                                if ghi <= q0:
                                    continue  # fully masked for this kb
                                lo = max(glo, q0)
                                nc.tensor.matmul(
                                    pvt[g][:, lo - glo:512],
                                    lhsT=v_sb[hv][kb][:, 0:HD + 1],
                                    rhs=pt[:, lo:ghi],
                                    start=(kb == 0),
                                    stop=(kb == 4 * g + 3),
                                )
                    # normalize: attnT[d, q] = pv[d, q] / pv[64, q]
                    for g in range(4):
                        den = dvt.tile([1, 512], F32, tag="den", name="den")
                        # custom-DVE ops read from physical partition 0 —
                        # copy the denominator row down first.
                        nc.vector.tensor_copy(out=den, in_=pvt[g][HD:HD + 1, :])
                        rc = dvt.tile([1, 512], F32, tag="rc", name="rc")
                        # denominators are sums of exp() > 0 — approx_fast
                        # edge cases (0/denorm/inf) cannot occur.
                        nc.vector.reciprocal_approx_fast(out=rc, in_=den)
                        rcb = dvt.tile([HD, 512], F32, tag="rcb", name="rcb")
                        nc.gpsimd.partition_broadcast(rcb, rc, channels=HD)
                        osl = slice(g * 512, (g + 1) * 512)
                        nc.vector.tensor_mul(
                            attnT_sb[qc][qoff:qoff + HD, osl], pvt[g][0:HD, :], rcb
                        )

            # ---------------- Phase 3: partial o = attn @ w_o_part
            OCH = 512
            with (
                tc.tile_pool(name="p3ps", bufs=2, space="PSUM") as p3ps,
                tc.tile_pool(name="osb", bufs=3) as osb,
            ):
                for qb in range(S // P):
                    for nch in range(H // OCH):
                        po = p3ps.tile([P, OCH], F32, tag="po")
                        for c in range(4):
                            nc.tensor.matmul(
                                po,
                                lhsT=attnT_sb[c][:, qb * P:(qb + 1) * P],
                                rhs=wo_all[:, c, nch * OCH:(nch + 1) * OCH],
                                start=(c == 0),
                                stop=(c == 3),
                            )
                        ot = osb.tile([P, OCH], F32, tag="ot")
                        nc.any.tensor_copy(out=ot, in_=po)
                        nc.sync.dma_start(
                            out=out[qb * P:(qb + 1) * P, nch * OCH:(nch + 1) * OCH],
                            in_=ot,
                        )

    nc.compile()
    return nc


def _host_tables():
    inv = (1.0 / ROPE_BASE ** (np.arange(0, HD, 2) / HD)).astype(np.float64)  # [32]
    ang = np.arange(S, dtype=np.float64)[:, None] * inv[None, :]  # [S, 32]
    cos32 = np.cos(ang).T.astype(np.float32)  # [32, S]
    sin32 = np.sin(ang).T.astype(np.float32)
    cosx = np.tile(cos32, (4, 1))  # [128, S]
    # sign-folded for the swap32 formulation: -sin on even halves, +sin on odd
    sinx = np.concatenate([-sin32, sin32, -sin32, sin32], axis=0)
    tri = (np.arange(P)[None, :] >= np.arange(P)[:, None]).astype(np.float16)
    idn = np.concatenate([np.eye(64, dtype=np.float16)] * 2, axis=0)  # [128, 64]
    return cosx, sinx, tri, idn


_PERM = np.concatenate([np.arange(0, HD, 2), np.arange(1, HD, 2)])  # deinterleave


def make_in_maps(x, w_qkv, w_o):
    """Build the 8 per-core input maps from full inputs."""
    cosx, sinx, tri, idn = _host_tables()
    in_maps = []
    for c in range(8):
        b, g = c // 4, c % 4
        xTc = np.ascontiguousarray(x[b].T).astype(np.float16)
        cols = []
        for hq in range(NHL * g, NHL * (g + 1)):
            cols.append(hq * HD + _PERM)
        qcols = np.concatenate(cols)
        cols = []
        for kv in range(NKVL * g, NKVL * (g + 1)):
            cols.append(H + kv * HD + _PERM)
        kcols = np.concatenate(cols)
        cols = []
        for kv in range(NKVL * g, NKVL * (g + 1)):
            cols.append(H + NKV * HD + kv * HD + np.arange(HD))
        vcols = np.concatenate(cols)
        wc = np.concatenate(
            [w_qkv[:, qcols], w_qkv[:, kcols], w_qkv[:, vcols]], axis=1
        ).astype(np.float16)
        woc = w_o[FQ * g:FQ * (g + 1), :].astype(np.float16)
        in_maps.append(
            {
                "xT": xTc,
                "wqkv": wc,
                "wo": woc,
                "cosx": cosx,
                "sinx": sinx,
                "tri": tri,
                "idn": idn,
                "ones": np.ones((P, 1), dtype=np.float16),
            }
        )
    return in_maps


_NC = None


def get_nc():
    global _NC
    if _NC is None:
        _NC = build_bass()
    return _NC


def kernel(x, mask, w_qkv, w_o):
    x = np.asarray(x)
    w_qkv = np.asarray(w_qkv)
    w_o = np.asarray(w_o)
    nc = get_nc()
    in_maps = make_in_maps(x, w_qkv, w_o)
    res = run_bass_kernel_spmd(nc, in_maps, core_ids=list(range(8)))
    out = np.zeros((2, S, H), dtype=np.float32)
    for c in range(8):
        out[c // 4] += res.results[c]["out"]
    return out


# revision 34
# speedup vs baseline: 1.3524x; 1.0745x over previous
"""Trainium2 Bass kernel for GQA attention block (B=2, S=2048, H=2048, NH=32, NKV=8, HD=64).

Sharding: 8 cores = data-parallel over batch (2) x tensor-parallel over heads (4).
Each core computes the qkv projection for its 8 q-heads / 2 kv-heads, RoPE,
causal GQA attention, and a partial o-projection (its 512 rows of w_o). The
host sums the 4 partial outputs per batch.

Device-side design:
  - x is passed pre-transposed (xT [H, S]) so the hidden dim lives on SBUF
    partitions for the qkv matmuls, producing qkv^T [feat, seq] directly.
  - Interleaved RoPE becomes contiguous half-swaps via a host-side column
    permutation of w_qkv (even dims first, odd dims second).
  - Scores are computed transposed (k on partitions, q free); the softmax
    denominator comes from a ones-row appended to v in the probs@v matmul:
    no probs transpose, no reduction pass.
  - exp() skips max-subtraction (scores ~N(0,1), mathematically identical);
    the 1/sqrt(HD) scale rides the ACT activation's free `scale`.
  - All matmul operands are float16 (10-bit mantissa = TF32-grade accuracy,
    1 cycle/row at any N, overlapped fast weight loads). PSUM accumulation
    stays fp32; score PSUM tiles are fp16 (1024 cols/bank) so one score
    matmul chunk + one exp instruction cover a whole key block.
  - Each 512-wide PV accumulator owns a full PSUM bank (start=True clears
    has_written for the whole bank, so groups must never share one).
"""

import sys

if "/opt/trn_rl_repo" not in sys.path:
    sys.path.insert(0, "/opt/trn_rl_repo")

import numpy as np

import concourse.bass as bass
import concourse.mybir as mybir
import concourse.tile as tile
from concourse import bacc
from concourse.bass_utils import run_bass_kernel_spmd

P = 128
S = 2048
H = 2048
NH = 32
NKV = 8
HD = 64
GROUPS = NH // NKV  # 4
NHL = 8   # local q heads per core
NKVL = 2  # local kv heads per core
FQ = NHL * HD   # 512
F = FQ + 2 * NKVL * HD  # 768
NKB = S // P    # 16 key blocks
ROPE_BASE = 10000.0

F32 = mybir.dt.float32
F16 = mybir.dt.float16


def build_bass():
    nc = bacc.Bacc("TRN2", num_devices=8)

    xT = nc.declare_dram_parameter("xT", [H, S], F16, isOutput=False)
    wqkv = nc.declare_dram_parameter("wqkv", [H, F], F16, isOutput=False)
    wo = nc.declare_dram_parameter("wo", [FQ, H], F16, isOutput=False)
    cosx = nc.declare_dram_parameter("cosx", [P, S], F32, isOutput=False)
    sinx = nc.declare_dram_parameter("sinx", [P, S], F32, isOutput=False)
    tri = nc.declare_dram_parameter("tri", [P, P], F16, isOutput=False)
    idn = nc.declare_dram_parameter("idn", [P, 64], F16, isOutput=False)
    ones = nc.declare_dram_parameter("ones", [P, 1], F16, isOutput=False)
    out = nc.declare_dram_parameter("out", [S, H], F32, isOutput=True)

    with tile.TileContext(nc) as tc:
        with (
            tc.tile_pool(name="const", bufs=1) as const,
            tc.tile_pool(name="wq", bufs=1) as wqp,
            tc.tile_pool(name="qkvT", bufs=1) as qkvp,
            tc.tile_pool(name="vsb", bufs=1) as vsbp,
            tc.tile_pool(name="attnT", bufs=1) as attp,
            tc.tile_pool(name="wop", bufs=1) as wop,
        ):
            cos_sb = const.tile([P, S], F32)
            sin_sb = const.tile([P, S], F32)
            tri_sb = const.tile([P, P], F16)
            idn_sb = const.tile([P, 64], F16)
            nc.sync.dma_start(out=cos_sb, in_=cosx.ap())
            nc.sync.dma_start(out=sin_sb, in_=sinx.ap())
            nc.sync.dma_start(out=tri_sb, in_=tri.ap())
            nc.sync.dma_start(out=idn_sb, in_=idn.ap())

            # weights resident: wq_all [128, h-chunk, feat], wo_all [128, c, H]
            wq_all = wqp.tile([P, H // P, F], F16)
            nc.sync.dma_start(
                out=wq_all, in_=wqkv.ap().rearrange("(ho p) f -> p ho f", p=P)
            )
            wo_all = wop.tile([P, 4, H], F16)
            nc.sync.dma_start(
                out=wo_all, in_=wo.ap().rearrange("(c p) n -> p c n", p=P)
            )

            # qkv^T: 4 q chunks (2 heads each); k replicated per kv head at
            # both 64-partition offsets (matmul operands must share their
            # base partition).
            qT_sb = [qkvp.tile([P, S], F16, tag=f"qT{c}", name=f"qT{c}") for c in range(4)]
            kT_rep = [qkvp.tile([P, S], F16, tag=f"kT{h}", name=f"kT{h}") for h in range(NKVL)]
            # v in [seq, hd] layout per kv head and key block, + ones column
            v_sb = [
                [vsbp.tile([P, HD + 1], F16, tag=f"v{hv}_{kb}", name=f"v{hv}_{kb}") for kb in range(NKB)]
                for hv in range(NKVL)
            ]
            attnT_sb = [attp.tile([P, S], F16, tag=f"at{c}", name=f"at{c}") for c in range(4)]

            # ------- PE warmup: ~4.5us of dummy matmuls so the HAM clock
            # gate ramps to 2.4 GHz before the real work arrives.
            with tc.tile_pool(name="wup", bufs=1, space="PSUM") as wupp:
                wup = wupp.tile([P, P], F32)
                for _ in range(40):
                    nc.tensor.matmul(wup, lhsT=tri_sb, rhs=tri_sb, start=True, stop=True)

            # ---------------- Phase 1: qkv^T = wqkv^T @ x^T, RoPE, v transpose
            SCH = 512  # seq chunk width for qkv matmuls
            with (
                tc.tile_pool(name="xw", bufs=2) as xw,
                tc.tile_pool(name="p1ps", bufs=3, space="PSUM") as p1ps,
                tc.tile_pool(name="p1vt", bufs=2, space="PSUM") as p1vt,
                tc.tile_pool(name="rtmp", bufs=3) as rtmp,
                tc.tile_pool(name="vtt", bufs=2) as vtt,
            ):
                for s in range(S // SCH):
                    ssl = slice(s * SCH, (s + 1) * SCH)
                    # x tiles for this seq chunk stay resident across the six
                    # feature jobs (per-h tags, double-buffered across s)
                    xts = []
                    for h in range(H // P):
                        xt = xw.tile([P, SCH], F16, tag=f"xt{h}", name=f"xt{h}")
                        nc.sync.dma_start(out=xt, in_=xT[h * P:(h + 1) * P, ssl])
                        xts.append(xt)
                    for f in range(6):
                        # shared tag + bufs=3: feature job f+1's accumulation
                        # overlaps job f's RoPE/copy consumption
                        t = p1ps.tile([P, SCH], F32, tag="qkv", name="qkv")
                        for h in range(H // P):
                            nc.tensor.matmul(
                                t,
                                lhsT=wq_all[:, h, f * P:(f + 1) * P],
                                rhs=xts[h],
                                start=(h == 0),
                                stop=(h == H // P - 1),
                            )
                        if f < 5:
                            # RoPE, full-width DVE ops:
                            #   out = t * cos + swap32(t) * sin_signed
                            # swap32 exchanges adjacent 32-row blocks; the
                            # swapped copy is made by the otherwise-idle ACT
                            # engine (it can read PSUM).
                            tsw = rtmp.tile([P, SCH], F32, tag="tsw", name="tsw")
                            for b in range(4):
                                src = (b ^ 1) * 32
                                nc.scalar.copy(
                                    out=tsw[b * 32:(b + 1) * 32, :],
                                    in_=t[src:src + 32, :],
                                )
                            ta = rtmp.tile([P, SCH], F32, tag="ta", name="ta")
                            nc.vector.tensor_mul(ta, t, cos_sb[:, ssl])
                            nc.vector.tensor_mul(tsw, tsw, sin_sb[:, ssl])
                            if f < 4:
                                nc.vector.tensor_add(qT_sb[f][:, ssl], ta, tsw)
                            else:
                                # k rows: 0:64 = kv0, 64:128 = kv1; write each
                                # kv head at both 64-partition offsets
                                for hh in range(2):
                                    si = slice(hh * 64, hh * 64 + 64)
                                    nc.vector.tensor_add(
                                        kT_rep[hh][0:64, ssl], ta[si, :], tsw[si, :]
                                    )
                                    nc.vector.tensor_add(
                                        kT_rep[hh][64:128, ssl], ta[si, :], tsw[si, :]
                                    )
                        else:
                            # v^T [2 kv heads x 64, SCH] -> [seq, hd] tiles
                            vt = vtt.tile([P, SCH], F16, tag="vt")
                            nc.vector.tensor_copy(out=vt, in_=t)
                            for hv in range(NKVL):
                                for j in range(SCH // P):
                                    kb = (s * SCH + j * P) // P
                                    pvtr = p1vt.tile([P, HD], F16, tag="vtp", name="vtp")
                                    nc.tensor.transpose(
                                        pvtr,
                                        vt[hv * HD:(hv + 1) * HD, j * P:(j + 1) * P],
                                        idn_sb[hv * HD:(hv + 1) * HD, :],
                                    )
                                    nc.vector.tensor_copy(
                                        out=v_sb[hv][kb][:, 0:HD], in_=pvtr
                                    )
                                    nc.sync.dma_start(
                                        out=v_sb[hv][kb][:, HD:HD + 1], in_=ones.ap()
                                    )

            # ---------------- Phase 2: attention, one pass per head.
            # Score PSUM is fp16 (1024 cols/bank): one matmul chunk per bank,
            # one exp per (head, key block). PV: 4 fp32 accumulators of 512
            # columns, one full bank each.
            with (
                tc.tile_pool(name="probs", bufs=3) as prp,
                tc.tile_pool(name="p2sc", bufs=2, space="PSUM") as p2sc,
                tc.tile_pool(name="p2pv", bufs=1, space="PSUM") as p2pv,
                tc.tile_pool(name="dvt", bufs=2) as dvt,
            ):
                for hl in range(NHL):
                    qc, qoff = hl // 2, (hl % 2) * HD
                    hv = hl // GROUPS
                    kTh = kT_rep[hv][qoff:qoff + HD, :]
                    qTh = qT_sb[qc][qoff:qoff + HD, :]
                    pvt = [
                        p2pv.tile([HD + 1, 512], F32, tag=f"pv{g}", name=f"pv{g}")
                        for g in range(4)
                    ]
                    # software-pipelined job list: one job per 1024-col score
                    # chunk. The QK matmuls for job j+1 are emitted BEFORE
                    # exp/PV of job j so the in-order PE stream never sits
                    # behind a PV that is waiting on the ACT exp.
                    jobs = []
                    for kb in range(NKB):
                        ch0 = kb * P
                        while ch0 < S:
                            cw = min(1024 - ch0 % 1024, S - ch0)
                            jobs.append((kb, ch0, cw))
                            ch0 += cw

                    def emit_qk(job):
                        kb, ch0, cw = job
                        q0 = kb * P
                        sc = p2sc.tile([P, 1024], F32, tag="sc")
                        mm0 = 0
                        while mm0 < cw:
                            mw = min(512, cw - mm0)
                            nc.tensor.matmul(
                                sc[:, mm0:mm0 + mw],
                                lhsT=kTh[:, q0:q0 + P],
                                rhs=qTh[:, ch0 + mm0:ch0 + mm0 + mw],
                                start=True,
                                stop=True,
                            )
                            mm0 += mw
                        return sc

                    sc_pending = emit_qk(jobs[0])
                    pt = None
                    for idx, job in enumerate(jobs):
                        kb, ch0, cw = job
                        q0 = kb * P
                        sc = sc_pending
                        if idx + 1 < len(jobs):
                            sc_pending = emit_qk(jobs[idx + 1])
                        if ch0 == q0:  # first chunk of this key block
                            pt = prp.tile([P, S], F16, tag="pt", name="pt")
                        # exp(score/8): the 1/sqrt(HD) scale rides the ACT op
                        nc.scalar.activation(
                            out=pt[:, ch0:ch0 + cw],
                            in_=sc[:, 0:cw],
                            func=mybir.ActivationFunctionType.Exp,
                            scale=0.125,
                        )
                        if ch0 == q0:
                            # causal mask on the diagonal block (mult after exp)
                            nc.vector.tensor_mul(
                                pt[:, q0:q0 + P], pt[:, q0:q0 + P], tri_sb
                            )
                        if ch0 + cw == S:  # last chunk: PV for this key block
                            for g in range(4):
                                glo, ghi = g * 512, (g + 1) * 512
                                if ghi <= q0:
                                    continue  # fully masked for this kb
                                lo = max(glo, q0)
                                nc.tensor.matmul(
                                    pvt[g][:, lo - glo:512],
                                    lhsT=v_sb[hv][kb][:, 0:HD + 1],
                                    rhs=pt[:, lo:ghi],
                                    start=(kb == 0),
                                    stop=(kb == 4 * g + 3),
                                )
                    # normalize: attnT[d, q] = pv[d, q] / pv[64, q]
                    for g in range(4):
                        den = dvt.tile([1, 512], F32, tag="den", name="den")
                        # custom-DVE ops read from physical partition 0 —
                        # copy the denominator row down first.
                        nc.vector.tensor_copy(out=den, in_=pvt[g][HD:HD + 1, :])
                        rc = dvt.tile([1, 512], F32, tag="rc", name="rc")
                        # denominators are sums of exp() > 0 — approx_fast
                        # edge cases (0/denorm/inf) cannot occur.
                        nc.vector.reciprocal_approx_fast(out=rc, in_=den)
                        rcb = dvt.tile([HD, 512], F32, tag="rcb", name="rcb")
                        nc.gpsimd.partition_broadcast(rcb, rc, channels=HD)
                        osl = slice(g * 512, (g + 1) * 512)
                        nc.vector.tensor_mul(
                            attnT_sb[qc][qoff:qoff + HD, osl], pvt[g][0:HD, :], rcb
                        )

            # ---------------- Phase 3: partial o = attn @ w_o_part
            OCH = 512
            with (
                tc.tile_pool(name="p3ps", bufs=2, space="PSUM") as p3ps,
                tc.tile_pool(name="osb", bufs=3) as osb,
            ):
                for qb in range(S // P):
                    for nch in range(H // OCH):
                        po = p3ps.tile([P, OCH], F32, tag="po")
                        for c in range(4):
                            nc.tensor.matmul(
                                po,
                                lhsT=attnT_sb[c][:, qb * P:(qb + 1) * P],
                                rhs=wo_all[:, c, nch * OCH:(nch + 1) * OCH],
                                start=(c == 0),
                                stop=(c == 3),
                            )
                        ot = osb.tile([P, OCH], F32, tag="ot")
                        nc.any.tensor_copy(out=ot, in_=po)
                        nc.sync.dma_start(
                            out=out[qb * P:(qb + 1) * P, nch * OCH:(nch + 1) * OCH],
                            in_=ot,
                        )

    nc.compile()
    return nc


def _host_tables():
    inv = (1.0 / ROPE_BASE ** (np.arange(0, HD, 2) / HD)).astype(np.float64)  # [32]
    ang = np.arange(S, dtype=np.float64)[:, None] * inv[None, :]  # [S, 32]
    cos32 = np.cos(ang).T.astype(np.float32)  # [32, S]
    sin32 = np.sin(ang).T.astype(np.float32)
    cosx = np.tile(cos32, (4, 1))  # [128, S]
    # sign-folded for the swap32 formulation: -sin on even halves, +sin on odd
    sinx = np.concatenate([-sin32, sin32, -sin32, sin32], axis=0)
    tri = (np.arange(P)[None, :] >= np.arange(P)[:, None]).astype(np.float16)
    idn = np.concatenate([np.eye(64, dtype=np.float16)] * 2, axis=0)  # [128, 64]
    return cosx, sinx, tri, idn


_PERM = np.concatenate([np.arange(0, HD, 2), np.arange(1, HD, 2)])  # deinterleave


def make_in_maps(x, w_qkv, w_o):
    """Build the 8 per-core input maps from full inputs."""
    cosx, sinx, tri, idn = _host_tables()
    in_maps = []
    for c in range(8):
        b, g = c // 4, c % 4
        xTc = np.ascontiguousarray(x[b].T).astype(np.float16)
        cols = []
        for hq in range(NHL * g, NHL * (g + 1)):
            cols.append(hq * HD + _PERM)
        qcols = np.concatenate(cols)
        cols = []
        for kv in range(NKVL * g, NKVL * (g + 1)):
            cols.append(H + kv * HD + _PERM)
        kcols = np.concatenate(cols)
        cols = []
        for kv in range(NKVL * g, NKVL * (g + 1)):
            cols.append(H + NKV * HD + kv * HD + np.arange(HD))
        vcols = np.concatenate(cols)
        wc = np.concatenate(
            [w_qkv[:, qcols], w_qkv[:, kcols], w_qkv[:, vcols]], axis=1
        ).astype(np.float16)
        woc = w_o[FQ * g:FQ * (g + 1), :].astype(np.float16)
        in_maps.append(
            {
                "xT": xTc,
                "wqkv": wc,
                "wo": woc,
                "cosx": cosx,
                "sinx": sinx,
                "tri": tri,
                "idn": idn,
                "ones": np.ones((P, 1), dtype=np.float16),
            }
        )
    return in_maps


_NC = None


def get_nc():
    global _NC
    if _NC is None:
        _NC = build_bass()
    return _NC


def kernel(x, mask, w_qkv, w_o):
    x = np.asarray(x)
    w_qkv = np.asarray(w_qkv)
    w_o = np.asarray(w_o)
    nc = get_nc()
    in_maps = make_in_maps(x, w_qkv, w_o)
    res = run_bass_kernel_spmd(nc, in_maps, core_ids=list(range(8)))
    out = np.zeros((2, S, H), dtype=np.float32)
    for c in range(8):
        out[c // 4] += res.results[c]["out"]
    return out


# revision 35
# speedup vs baseline: 1.3746x; 1.0164x over previous
"""Trainium2 Bass kernel for GQA attention block (B=2, S=2048, H=2048, NH=32, NKV=8, HD=64).

Sharding: 8 cores = data-parallel over batch (2) x tensor-parallel over heads (4).
Each core computes the qkv projection for its 8 q-heads / 2 kv-heads, RoPE,
causal GQA attention, and a partial o-projection (its 512 rows of w_o). The
host sums the 4 partial outputs per batch.

Device-side design:
  - x is passed pre-transposed (xT [H, S]) so the hidden dim lives on SBUF
    partitions for the qkv matmuls, producing qkv^T [feat, seq] directly.
  - Interleaved RoPE becomes contiguous half-swaps via a host-side column
    permutation of w_qkv (even dims first, odd dims second).
  - Scores are computed transposed (k on partitions, q free); the softmax
    denominator comes from a ones-row appended to v in the probs@v matmul:
    no probs transpose, no reduction pass.
  - exp() skips max-subtraction (scores ~N(0,1), mathematically identical);
    the 1/sqrt(HD) scale rides the ACT activation's free `scale`.
  - All matmul operands are float16 (10-bit mantissa = TF32-grade accuracy,
    1 cycle/row at any N, overlapped fast weight loads). PSUM accumulation
    stays fp32; score PSUM tiles are fp16 (1024 cols/bank) so one score
    matmul chunk + one exp instruction cover a whole key block.
  - Each 512-wide PV accumulator owns a full PSUM bank (start=True clears
    has_written for the whole bank, so groups must never share one).
"""

import sys

if "/opt/trn_rl_repo" not in sys.path:
    sys.path.insert(0, "/opt/trn_rl_repo")

import numpy as np

import concourse.bass as bass
import concourse.mybir as mybir
import concourse.tile as tile
from concourse import bacc
from concourse.bass_utils import run_bass_kernel_spmd

P = 128
S = 2048
H = 2048
NH = 32
NKV = 8
HD = 64
GROUPS = NH // NKV  # 4
NHL = 8   # local q heads per core
NKVL = 2  # local kv heads per core
FQ = NHL * HD   # 512
F = FQ + 2 * NKVL * HD  # 768
NKB = S // P    # 16 key blocks
ROPE_BASE = 10000.0

F32 = mybir.dt.float32
F16 = mybir.dt.float16


def build_bass():
    nc = bacc.Bacc("TRN2", num_devices=8)

    xT = nc.declare_dram_parameter("xT", [H, S], F16, isOutput=False)
    wqkv = nc.declare_dram_parameter("wqkv", [H, F], F16, isOutput=False)
    wo = nc.declare_dram_parameter("wo", [FQ, H], F16, isOutput=False)
    cosx = nc.declare_dram_parameter("cosx", [P, S], F32, isOutput=False)
    sinx = nc.declare_dram_parameter("sinx", [P, S], F32, isOutput=False)
    tri = nc.declare_dram_parameter("tri", [P, P], F16, isOutput=False)
    idn = nc.declare_dram_parameter("idn", [P, 64], F16, isOutput=False)
    ones = nc.declare_dram_parameter("ones", [P, 1], F16, isOutput=False)
    out = nc.declare_dram_parameter("out", [S, H], F32, isOutput=True)

    with tile.TileContext(nc) as tc:
        with (
            tc.tile_pool(name="const", bufs=1) as const,
            tc.tile_pool(name="wq", bufs=1) as wqp,
            tc.tile_pool(name="qkvT", bufs=1) as qkvp,
            tc.tile_pool(name="vsb", bufs=1) as vsbp,
            tc.tile_pool(name="attnT", bufs=1) as attp,
            tc.tile_pool(name="wop", bufs=1) as wop,
        ):
            cos_sb = const.tile([P, S], F32)
            sin_sb = const.tile([P, S], F32)
            tri_sb = const.tile([P, P], F16)
            idn_sb = const.tile([P, 64], F16)
            nc.sync.dma_start(out=cos_sb, in_=cosx.ap())
            nc.sync.dma_start(out=sin_sb, in_=sinx.ap())
            nc.sync.dma_start(out=tri_sb, in_=tri.ap())
            nc.sync.dma_start(out=idn_sb, in_=idn.ap())

            # weights resident: wq_all [128, h-chunk, feat], wo_all [128, c, H]
            wq_all = wqp.tile([P, H // P, F], F16)
            nc.sync.dma_start(
                out=wq_all, in_=wqkv.ap().rearrange("(ho p) f -> p ho f", p=P)
            )
            wo_all = wop.tile([P, 4, H], F16)
            nc.sync.dma_start(
                out=wo_all, in_=wo.ap().rearrange("(c p) n -> p c n", p=P)
            )

            # qkv^T: 4 q chunks (2 heads each); k replicated per kv head at
            # both 64-partition offsets (matmul operands must share their
            # base partition).
            qT_sb = [qkvp.tile([P, S], F16, tag=f"qT{c}", name=f"qT{c}") for c in range(4)]
            kT_rep = [qkvp.tile([P, S], F16, tag=f"kT{h}", name=f"kT{h}") for h in range(NKVL)]
            # v in [seq, hd] layout per kv head and key block, + ones column
            v_sb = [
                [vsbp.tile([P, HD + 1], F16, tag=f"v{hv}_{kb}", name=f"v{hv}_{kb}") for kb in range(NKB)]
                for hv in range(NKVL)
            ]
            attnT_sb = [attp.tile([P, S], F16, tag=f"at{c}", name=f"at{c}") for c in range(4)]

            # ------- PE warmup: ~4.5us of dummy matmuls so the HAM clock
            # gate ramps to 2.4 GHz before the real work arrives.
            with tc.tile_pool(name="wup", bufs=1, space="PSUM") as wupp:
                wup = wupp.tile([P, P], F32)
                for _ in range(40):
                    nc.tensor.matmul(wup, lhsT=tri_sb, rhs=tri_sb, start=True, stop=True)

            # ---------------- Phase 1: qkv^T = wqkv^T @ x^T, RoPE, v transpose
            SCH = 512  # seq chunk width for qkv matmuls
            with (
                tc.tile_pool(name="xw", bufs=2) as xw,
                tc.tile_pool(name="p1ps", bufs=3, space="PSUM") as p1ps,
                tc.tile_pool(name="p1vt", bufs=2, space="PSUM") as p1vt,
                tc.tile_pool(name="rtmp", bufs=3) as rtmp,
                tc.tile_pool(name="vtt", bufs=2) as vtt,
            ):
                for s in range(S // SCH):
                    ssl = slice(s * SCH, (s + 1) * SCH)
                    # x tiles for this seq chunk stay resident across the six
                    # feature jobs (per-h tags, double-buffered across s)
                    xts = []
                    for h in range(H // P):
                        xt = xw.tile([P, SCH], F16, tag=f"xt{h}", name=f"xt{h}")
                        nc.sync.dma_start(out=xt, in_=xT[h * P:(h + 1) * P, ssl])
                        xts.append(xt)
                    for f in range(6):
                        # shared tag + bufs=3: feature job f+1's accumulation
                        # overlaps job f's RoPE/copy consumption
                        t = p1ps.tile([P, SCH], F32, tag="qkv", name="qkv")
                        for h in range(H // P):
                            nc.tensor.matmul(
                                t,
                                lhsT=wq_all[:, h, f * P:(f + 1) * P],
                                rhs=xts[h],
                                start=(h == 0),
                                stop=(h == H // P - 1),
                            )
                        if f < 5:
                            # RoPE, full-width DVE ops:
                            #   out = t * cos + swap32(t) * sin_signed
                            # swap32 exchanges adjacent 32-row blocks; the
                            # swapped copy is made by the otherwise-idle ACT
                            # engine (it can read PSUM).
                            tsw = rtmp.tile([P, SCH], F32, tag="tsw", name="tsw")
                            for b in range(4):
                                src = (b ^ 1) * 32
                                nc.scalar.copy(
                                    out=tsw[b * 32:(b + 1) * 32, :],
                                    in_=t[src:src + 32, :],
                                )
                            ta = rtmp.tile([P, SCH], F32, tag="ta", name="ta")
                            nc.vector.tensor_mul(ta, t, cos_sb[:, ssl])
                            nc.vector.tensor_mul(tsw, tsw, sin_sb[:, ssl])
                            if f < 4:
                                nc.vector.tensor_add(qT_sb[f][:, ssl], ta, tsw)
                            else:
                                # k rows: 0:64 = kv0, 64:128 = kv1; write each
                                # kv head at both 64-partition offsets
                                for hh in range(2):
                                    si = slice(hh * 64, hh * 64 + 64)
                                    nc.vector.tensor_add(
                                        kT_rep[hh][0:64, ssl], ta[si, :], tsw[si, :]
                                    )
                                    nc.vector.tensor_add(
                                        kT_rep[hh][64:128, ssl], ta[si, :], tsw[si, :]
                                    )
                        else:
                            # v^T [2 kv heads x 64, SCH] -> [seq, hd] tiles
                            vt = vtt.tile([P, SCH], F16, tag="vt")
                            nc.vector.tensor_copy(out=vt, in_=t)
                            for hv in range(NKVL):
                                for j in range(SCH // P):
                                    kb = (s * SCH + j * P) // P
                                    pvtr = p1vt.tile([P, HD], F16, tag="vtp", name="vtp")
                                    nc.tensor.transpose(
                                        pvtr,
                                        vt[hv * HD:(hv + 1) * HD, j * P:(j + 1) * P],
                                        idn_sb[hv * HD:(hv + 1) * HD, :],
                                    )
                                    nc.vector.tensor_copy(
                                        out=v_sb[hv][kb][:, 0:HD], in_=pvtr
                                    )
                                    nc.sync.dma_start(
                                        out=v_sb[hv][kb][:, HD:HD + 1], in_=ones.ap()
                                    )

            # ---------------- Phase 2: attention, one pass per head.
            # Score PSUM is fp16 (1024 cols/bank): one matmul chunk per bank,
            # one exp per (head, key block). PV: 4 fp32 accumulators of 512
            # columns, one full bank each.
            with (
                tc.tile_pool(name="probs", bufs=4) as prp,
                tc.tile_pool(name="p2sc", bufs=2, space="PSUM") as p2sc,
                tc.tile_pool(name="p2pv", bufs=1, space="PSUM") as p2pv,
                tc.tile_pool(name="dvt", bufs=2) as dvt,
            ):
                for hl in range(NHL):
                    qc, qoff = hl // 2, (hl % 2) * HD
                    hv = hl // GROUPS
                    kTh = kT_rep[hv][qoff:qoff + HD, :]
                    qTh = qT_sb[qc][qoff:qoff + HD, :]
                    pvt = [
                        p2pv.tile([HD + 1, 512], F32, tag=f"pv{g}", name=f"pv{g}")
                        for g in range(4)
                    ]
                    # software-pipelined job list: one job per 1024-col score
                    # chunk. The QK matmuls for job j+1 are emitted BEFORE
                    # exp/PV of job j so the in-order PE stream never sits
                    # behind a PV that is waiting on the ACT exp.
                    jobs = []
                    for kb in range(NKB):
                        ch0 = kb * P
                        while ch0 < S:
                            cw = min(1024 - ch0 % 1024, S - ch0)
                            jobs.append((kb, ch0, cw))
                            ch0 += cw

                    def emit_qk(job):
                        kb, ch0, cw = job
                        q0 = kb * P
                        sc = p2sc.tile([P, 1024], F32, tag="sc")
                        mm0 = 0
                        while mm0 < cw:
                            mw = min(512, cw - mm0)
                            nc.tensor.matmul(
                                sc[:, mm0:mm0 + mw],
                                lhsT=kTh[:, q0:q0 + P],
                                rhs=qTh[:, ch0 + mm0:ch0 + mm0 + mw],
                                start=True,
                                stop=True,
                            )
                            mm0 += mw
                        return sc

                    sc_pending = emit_qk(jobs[0])
                    pt = None
                    for idx, job in enumerate(jobs):
                        kb, ch0, cw = job
                        q0 = kb * P
                        sc = sc_pending
                        if idx + 1 < len(jobs):
                            sc_pending = emit_qk(jobs[idx + 1])
                        if ch0 == q0:  # first chunk of this key block
                            pt = prp.tile([P, S], F16, tag="pt", name="pt")
                        # exp(score/8): the 1/sqrt(HD) scale rides the ACT op
                        nc.scalar.activation(
                            out=pt[:, ch0:ch0 + cw],
                            in_=sc[:, 0:cw],
                            func=mybir.ActivationFunctionType.Exp,
                            scale=0.125,
                        )
                        if ch0 == q0:
                            # causal mask on the diagonal block (mult after exp)
                            nc.vector.tensor_mul(
                                pt[:, q0:q0 + P], pt[:, q0:q0 + P], tri_sb
                            )
                        if ch0 + cw == S:  # last chunk: PV for this key block
                            for g in range(4):
                                glo, ghi = g * 512, (g + 1) * 512
                                if ghi <= q0:
                                    continue  # fully masked for this kb
                                lo = max(glo, q0)
                                nc.tensor.matmul(
                                    pvt[g][:, lo - glo:512],
                                    lhsT=v_sb[hv][kb][:, 0:HD + 1],
                                    rhs=pt[:, lo:ghi],
                                    start=(kb == 0),
                                    stop=(kb == 4 * g + 3),
                                )
                    # normalize: attnT[d, q] = pv[d, q] / pv[64, q]
                    for g in range(4):
                        den = dvt.tile([1, 512], F32, tag="den", name="den")
                        # custom-DVE ops read from physical partition 0 —
                        # copy the denominator row down first.
                        nc.vector.tensor_copy(out=den, in_=pvt[g][HD:HD + 1, :])
                        rc = dvt.tile([1, 512], F32, tag="rc", name="rc")
                        # denominators are sums of exp() > 0 — approx_fast
                        # edge cases (0/denorm/inf) cannot occur.
                        nc.vector.reciprocal_approx_fast(out=rc, in_=den)
                        rcb = dvt.tile([HD, 512], F32, tag="rcb", name="rcb")
                        nc.gpsimd.partition_broadcast(rcb, rc, channels=HD)
                        osl = slice(g * 512, (g + 1) * 512)
                        nc.vector.tensor_mul(
                            attnT_sb[qc][qoff:qoff + HD, osl], pvt[g][0:HD, :], rcb
                        )

            # ---------------- Phase 3: partial o = attn @ w_o_part
            OCH = 512
            with (
                tc.tile_pool(name="p3ps", bufs=4, space="PSUM") as p3ps,
                tc.tile_pool(name="osb", bufs=4) as osb,
            ):
                for qb in range(S // P):
                    for nch in range(H // OCH):
                        po = p3ps.tile([P, OCH], F32, tag="po")
                        for c in range(4):
                            nc.tensor.matmul(
                                po,
                                lhsT=attnT_sb[c][:, qb * P:(qb + 1) * P],
                                rhs=wo_all[:, c, nch * OCH:(nch + 1) * OCH],
                                start=(c == 0),
                                stop=(c == 3),
                            )
                        ot = osb.tile([P, OCH], F32, tag="ot")
                        nc.vector.tensor_copy(out=ot, in_=po)
                        nc.sync.dma_start(
                            out=out[qb * P:(qb + 1) * P, nch * OCH:(nch + 1) * OCH],
                            in_=ot,
                        )

    nc.compile()
    return nc


def _host_tables():
    inv = (1.0 / ROPE_BASE ** (np.arange(0, HD, 2) / HD)).astype(np.float64)  # [32]
    ang = np.arange(S, dtype=np.float64)[:, None] * inv[None, :]  # [S, 32]
    cos32 = np.cos(ang).T.astype(np.float32)  # [32, S]
    sin32 = np.sin(ang).T.astype(np.float32)
    cosx = np.tile(cos32, (4, 1))  # [128, S]
    # sign-folded for the swap32 formulation: -sin on even halves, +sin on odd
    sinx = np.concatenate([-sin32, sin32, -sin32, sin32], axis=0)
    tri = (np.arange(P)[None, :] >= np.arange(P)[:, None]).astype(np.float16)
    idn = np.concatenate([np.eye(64, dtype=np.float16)] * 2, axis=0)  # [128, 64]
    return cosx, sinx, tri, idn


_PERM = np.concatenate([np.arange(0, HD, 2), np.arange(1, HD, 2)])  # deinterleave


def make_in_maps(x, w_qkv, w_o):
    """Build the 8 per-core input maps from full inputs."""
    cosx, sinx, tri, idn = _host_tables()
    in_maps = []
    for c in range(8):
        b, g = c // 4, c % 4
        xTc = np.ascontiguousarray(x[b].T).astype(np.float16)
        cols = []
        for hq in range(NHL * g, NHL * (g + 1)):
            cols.append(hq * HD + _PERM)
        qcols = np.concatenate(cols)
        cols = []
        for kv in range(NKVL * g, NKVL * (g + 1)):
            cols.append(H + kv * HD + _PERM)
        kcols = np.concatenate(cols)
        cols = []
        for kv in range(NKVL * g, NKVL * (g + 1)):
            cols.append(H + NKV * HD + kv * HD + np.arange(HD))
        vcols = np.concatenate(cols)
        wc = np.concatenate(
            [w_qkv[:, qcols], w_qkv[:, kcols], w_qkv[:, vcols]], axis=1
        ).astype(np.float16)
        woc = w_o[FQ * g:FQ * (g + 1), :].astype(np.float16)
        in_maps.append(
            {
                "xT": xTc,
                "wqkv": wc,
                "wo": woc,
                "cosx": cosx,
                "sinx": sinx,
                "tri": tri,
                "idn": idn,
                "ones": np.ones((P, 1), dtype=np.float16),
            }
        )
    return in_maps


_NC = None


def get_nc():
    global _NC
    if _NC is None:
        _NC = build_bass()
    return _NC


def kernel(x, mask, w_qkv, w_o):
    x = np.asarray(x)
    w_qkv = np.asarray(w_qkv)
    w_o = np.asarray(w_o)
    nc = get_nc()
    in_maps = make_in_maps(x, w_qkv, w_o)
    res = run_bass_kernel_spmd(nc, in_maps, core_ids=list(range(8)))
    out = np.zeros((2, S, H), dtype=np.float32)
    for c in range(8):
        out[c // 4] += res.results[c]["out"]
    return out


# revision 37
# speedup vs baseline: 1.5907x; 1.1572x over previous
"""Trainium2 Bass kernel for GQA attention block (B=2, S=2048, H=2048, NH=32, NKV=8, HD=64).

Sharding: 8 cores = data-parallel over batch (2) x tensor-parallel over heads (4).
Each core computes the qkv projection for its 8 q-heads / 2 kv-heads, RoPE,
causal GQA attention, and a partial o-projection (its 512 rows of w_o). The
host sums the 4 partial outputs per batch.

Device-side design:
  - x is passed pre-transposed (xT [H, S]) so the hidden dim lives on SBUF
    partitions for the qkv matmuls, producing qkv^T [feat, seq] directly.
  - Interleaved RoPE becomes contiguous half-swaps via a host-side column
    permutation of w_qkv (even dims first, odd dims second).
  - Scores are computed transposed (k on partitions, q free); the softmax
    denominator comes from a ones-row appended to v in the probs@v matmul:
    no probs transpose, no reduction pass.
  - exp() skips max-subtraction (scores ~N(0,1), mathematically identical);
    the 1/sqrt(HD) scale rides the ACT activation's free `scale`.
  - All matmul operands are float16 (10-bit mantissa = TF32-grade accuracy,
    1 cycle/row at any N, overlapped fast weight loads). PSUM accumulation
    stays fp32; score PSUM tiles are fp16 (1024 cols/bank) so one score
    matmul chunk + one exp instruction cover a whole key block.
  - Each 512-wide PV accumulator owns a full PSUM bank (start=True clears
    has_written for the whole bank, so groups must never share one).
"""

import sys

if "/opt/trn_rl_repo" not in sys.path:
    sys.path.insert(0, "/opt/trn_rl_repo")

import numpy as np

import concourse.bass as bass
import concourse.mybir as mybir
import concourse.tile as tile
from concourse import bacc
from concourse.bass_utils import run_bass_kernel_spmd

P = 128
S = 2048
H = 2048
NH = 32
NKV = 8
HD = 64
GROUPS = NH // NKV  # 4
NHL = 8   # local q heads per core
NKVL = 2  # local kv heads per core
FQ = NHL * HD   # 512
F = FQ + 2 * NKVL * HD  # 768
NKB = S // P    # 16 key blocks
ROPE_BASE = 10000.0

F32 = mybir.dt.float32
F16 = mybir.dt.float16


def build_bass():
    nc = bacc.Bacc("TRN2", num_devices=8)

    xT = nc.declare_dram_parameter("xT", [H, S], F16, isOutput=False)
    wqkv = nc.declare_dram_parameter("wqkv", [H, F], F16, isOutput=False)
    wo = nc.declare_dram_parameter("wo", [FQ, H], F16, isOutput=False)
    cosx = nc.declare_dram_parameter("cosx", [P, S], F32, isOutput=False)
    sinx = nc.declare_dram_parameter("sinx", [P, S], F32, isOutput=False)
    tri = nc.declare_dram_parameter("tri", [P, P], F16, isOutput=False)
    idn = nc.declare_dram_parameter("idn", [P, 64], F16, isOutput=False)
    ones = nc.declare_dram_parameter("ones", [P, 1], F16, isOutput=False)
    out = nc.declare_dram_parameter("out", [S, H], F32, isOutput=True)

    with tile.TileContext(nc) as tc:
        with (
            tc.tile_pool(name="const", bufs=1) as const,
            tc.tile_pool(name="wq", bufs=1) as wqp,
            tc.tile_pool(name="qkvT", bufs=1) as qkvp,
            tc.tile_pool(name="vsb", bufs=1) as vsbp,
            tc.tile_pool(name="attnT", bufs=1) as attp,
            tc.tile_pool(name="wop", bufs=1) as wop,
        ):
            cos_sb = const.tile([P, S], F32)
            sin_sb = const.tile([P, S], F32)
            tri_sb = const.tile([P, P], F16)
            idn_sb = const.tile([P, 64], F16)
            nc.sync.dma_start(out=cos_sb, in_=cosx.ap())
            nc.sync.dma_start(out=sin_sb, in_=sinx.ap())
            nc.sync.dma_start(out=tri_sb, in_=tri.ap())
            nc.sync.dma_start(out=idn_sb, in_=idn.ap())

            # weights resident: wq_all [128, h-chunk, feat], wo_all [128, c, H]
            wq_all = wqp.tile([P, H // P, F], F16)
            nc.sync.dma_start(
                out=wq_all, in_=wqkv.ap().rearrange("(ho p) f -> p ho f", p=P)
            )
            wo_all = wop.tile([P, 4, H], F16)
            nc.sync.dma_start(
                out=wo_all, in_=wo.ap().rearrange("(c p) n -> p c n", p=P)
            )

            # qkv^T: 4 q chunks (2 heads each); k replicated per kv head at
            # both 64-partition offsets (matmul operands must share their
            # base partition).
            qT_sb = [qkvp.tile([P, S], F16, tag=f"qT{c}", name=f"qT{c}") for c in range(4)]
            kT_rep = [qkvp.tile([P, S], F16, tag=f"kT{h}", name=f"kT{h}") for h in range(NKVL)]
            # v in [seq, hd] layout per kv head and key block, + ones column
            v_sb = [
                [vsbp.tile([P, HD + 1], F16, tag=f"v{hv}_{kb}", name=f"v{hv}_{kb}") for kb in range(NKB)]
                for hv in range(NKVL)
            ]
            attnT_sb = [attp.tile([P, S], F16, tag=f"at{c}", name=f"at{c}") for c in range(4)]

            # ------- PE warmup: ~4.5us of dummy matmuls so the HAM clock
            # gate ramps to 2.4 GHz before the real work arrives.
            with tc.tile_pool(name="wup", bufs=1, space="PSUM") as wupp:
                wup = wupp.tile([P, P], F32)
                for _ in range(40):
                    nc.tensor.matmul(wup, lhsT=tri_sb, rhs=tri_sb, start=True, stop=True)

            # ---------------- Phase 1: qkv^T = wqkv^T @ x^T, RoPE, v transpose
            SCH = 512  # seq chunk width for qkv matmuls
            with (
                tc.tile_pool(name="xw", bufs=2) as xw,
                tc.tile_pool(name="p1ps", bufs=3, space="PSUM") as p1ps,
                tc.tile_pool(name="p1vt", bufs=2, space="PSUM") as p1vt,
                tc.tile_pool(name="rtmp", bufs=3) as rtmp,
                tc.tile_pool(name="vtt", bufs=2) as vtt,
            ):
                for s in range(S // SCH):
                    ssl = slice(s * SCH, (s + 1) * SCH)
                    # x tiles for this seq chunk stay resident across the six
                    # feature jobs (per-h tags, double-buffered across s)
                    xts = []
                    for h in range(H // P):
                        xt = xw.tile([P, SCH], F16, tag=f"xt{h}", name=f"xt{h}")
                        nc.sync.dma_start(out=xt, in_=xT[h * P:(h + 1) * P, ssl])
                        xts.append(xt)
                    for f in range(6):
                        # shared tag + bufs=3: feature job f+1's accumulation
                        # overlaps job f's RoPE/copy consumption
                        t = p1ps.tile([P, SCH], F32, tag="qkv", name="qkv")
                        for h in range(H // P):
                            nc.tensor.matmul(
                                t,
                                lhsT=wq_all[:, h, f * P:(f + 1) * P],
                                rhs=xts[h],
                                start=(h == 0),
                                stop=(h == H // P - 1),
                            )
                        if f < 5:
                            # RoPE, full-width DVE ops:
                            #   out = t * cos + swap32(t) * sin_signed
                            # swap32 exchanges adjacent 32-row blocks; the
                            # swapped copy is made by the otherwise-idle ACT
                            # engine (it can read PSUM).
                            tsw = rtmp.tile([P, SCH], F32, tag="tsw", name="tsw")
                            for b in range(4):
                                src = (b ^ 1) * 32
                                nc.scalar.copy(
                                    out=tsw[b * 32:(b + 1) * 32, :],
                                    in_=t[src:src + 32, :],
                                )
                            ta = rtmp.tile([P, SCH], F32, tag="ta", name="ta")
                            nc.vector.tensor_mul(ta, t, cos_sb[:, ssl])
                            nc.vector.tensor_mul(tsw, tsw, sin_sb[:, ssl])
                            if f < 4:
                                nc.vector.tensor_add(qT_sb[f][:, ssl], ta, tsw)
                            else:
                                # k rows: 0:64 = kv0, 64:128 = kv1; write each
                                # kv head at both 64-partition offsets
                                for hh in range(2):
                                    si = slice(hh * 64, hh * 64 + 64)
                                    nc.vector.tensor_add(
                                        kT_rep[hh][0:64, ssl], ta[si, :], tsw[si, :]
                                    )
                                    nc.vector.tensor_add(
                                        kT_rep[hh][64:128, ssl], ta[si, :], tsw[si, :]
                                    )
                        else:
                            # v^T [2 kv heads x 64, SCH] -> [seq, hd] tiles
                            vt = vtt.tile([P, SCH], F16, tag="vt")
                            nc.vector.tensor_copy(out=vt, in_=t)
                            for hv in range(NKVL):
                                for j in range(SCH // P):
                                    kb = (s * SCH + j * P) // P
                                    pvtr = p1vt.tile([P, HD], F16, tag="vtp", name="vtp")
                                    nc.tensor.transpose(
                                        pvtr,
                                        vt[hv * HD:(hv + 1) * HD, j * P:(j + 1) * P],
                                        idn_sb[hv * HD:(hv + 1) * HD, :],
                                    )
                                    nc.vector.tensor_copy(
                                        out=v_sb[hv][kb][:, 0:HD], in_=pvtr
                                    )
                                    nc.sync.dma_start(
                                        out=v_sb[hv][kb][:, HD:HD + 1], in_=ones.ap()
                                    )

            # ---------------- Phase 2: attention, one pass per head.
            # Score PSUM is fp16 (1024 cols/bank): one matmul chunk per bank,
            # one exp per (head, key block). PV: 4 fp32 accumulators of 512
            # columns, one full bank each.
            with (
                tc.tile_pool(name="probs", bufs=4) as prp,
                tc.tile_pool(name="p2sc", bufs=3, space="PSUM") as p2sc,
                tc.tile_pool(name="p2pv", bufs=1, space="PSUM") as p2pv,
                tc.tile_pool(name="dvt", bufs=2) as dvt,
            ):
                for hl in range(NHL):
                    qc, qoff = hl // 2, (hl % 2) * HD
                    hv = hl // GROUPS
                    kTh = kT_rep[hv][qoff:qoff + HD, :]
                    qTh = qT_sb[qc][qoff:qoff + HD, :]
                    # Two q-half passes per head: PV needs only 2 banks, which
                    # frees PSUM for a 3-deep score-tile pool so the QK
                    # prefetch runs 2 jobs ahead of exp/PV.
                    for pas in range(2):
                        qlo = pas * 1024
                        pvt = [
                            p2pv.tile([HD + 1, 512], F32, tag=f"pv{g}", name=f"pv{g}")
                            for g in range(2)
                        ]
                        jobs = []
                        for kb in range((pas + 1) * 8):
                            q0 = max(kb * P, qlo)
                            jobs.append((kb, q0, qlo + 1024 - q0))

                        def emit_qk(job):
                            kb, q0, W = job
                            sc = p2sc.tile([P, 1024], F32, tag="sc")
                            mm0 = 0
                            while mm0 < W:
                                mw = min(512, W - mm0)
                                nc.tensor.matmul(
                                    sc[:, mm0:mm0 + mw],
                                    lhsT=kTh[:, kb * P:(kb + 1) * P],
                                    rhs=qTh[:, q0 + mm0:q0 + mm0 + mw],
                                    start=True,
                                    stop=True,
                                )
                                mm0 += mw
                            return sc

                        pend = [emit_qk(jobs[0])]
                        if len(jobs) > 1:
                            pend.append(emit_qk(jobs[1]))
                        for idx, job in enumerate(jobs):
                            kb, q0, W = job
                            sc = pend.pop(0)
                            if idx + 2 < len(jobs):
                                pend.append(emit_qk(jobs[idx + 2]))
                            col0 = q0 - qlo
                            pt = prp.tile([P, 1024], F16, tag="pt", name="pt")
                            # exp(score/8): the 1/sqrt(HD) scale rides ACT
                            nc.scalar.activation(
                                out=pt[:, col0:col0 + W],
                                in_=sc[:, 0:W],
                                func=mybir.ActivationFunctionType.Exp,
                                scale=0.125,
                            )
                            if kb * P >= qlo:
                                # causal mask on the diagonal block
                                nc.vector.tensor_mul(
                                    pt[:, col0:col0 + P],
                                    pt[:, col0:col0 + P],
                                    tri_sb,
                                )
                            for g in range(2):
                                glo, ghi = g * 512, (g + 1) * 512
                                if ghi <= col0:
                                    continue  # fully masked for this kb
                                lo = max(glo, col0)
                                nc.tensor.matmul(
                                    pvt[g][:, lo - glo:512],
                                    lhsT=v_sb[hv][kb][:, 0:HD + 1],
                                    rhs=pt[:, lo:ghi],
                                    start=(kb == 0),
                                    stop=(kb == (qlo + ghi) // P - 1),
                                )
                        # normalize: attnT[d, q] = pv[d, q] / pv[64, q]
                        for g in range(2):
                            den = dvt.tile([1, 512], F32, tag="den", name="den")
                            # custom-DVE ops read from physical partition 0 —
                            # copy the denominator row down first.
                            nc.vector.tensor_copy(out=den, in_=pvt[g][HD:HD + 1, :])
                            rc = dvt.tile([1, 512], F32, tag="rc", name="rc")
                            # denominators are sums of exp() > 0 — approx_fast
                            # edge cases (0/denorm/inf) cannot occur.
                            nc.vector.reciprocal_approx_fast(out=rc, in_=den)
                            rcb = dvt.tile([HD, 512], F32, tag="rcb", name="rcb")
                            nc.gpsimd.partition_broadcast(rcb, rc, channels=HD)
                            osl = slice(qlo + g * 512, qlo + (g + 1) * 512)
                            nc.vector.tensor_mul(
                                attnT_sb[qc][qoff:qoff + HD, osl],
                                pvt[g][0:HD, :],
                                rcb,
                            )

            # ---------------- Phase 3: partial o = attn @ w_o_part
            OCH = 512
            with (
                tc.tile_pool(name="p3ps", bufs=4, space="PSUM") as p3ps,
                tc.tile_pool(name="osb", bufs=4) as osb,
            ):
                for qb in range(S // P):
                    for nch in range(H // OCH):
                        po = p3ps.tile([P, OCH], F32, tag="po")
                        for c in range(4):
                            nc.tensor.matmul(
                                po,
                                lhsT=attnT_sb[c][:, qb * P:(qb + 1) * P],
                                rhs=wo_all[:, c, nch * OCH:(nch + 1) * OCH],
                                start=(c == 0),
                                stop=(c == 3),
                            )
                        ot = osb.tile([P, OCH], F32, tag="ot")
                        nc.vector.tensor_copy(out=ot, in_=po)
                        nc.sync.dma_start(
                            out=out[qb * P:(qb + 1) * P, nch * OCH:(nch + 1) * OCH],
                            in_=ot,
                        )

    nc.compile()
    return nc


def _host_tables():
    inv = (1.0 / ROPE_BASE ** (np.arange(0, HD, 2) / HD)).astype(np.float64)  # [32]
    ang = np.arange(S, dtype=np.float64)[:, None] * inv[None, :]  # [S, 32]
    cos32 = np.cos(ang).T.astype(np.float32)  # [32, S]
    sin32 = np.sin(ang).T.astype(np.float32)
    cosx = np.tile(cos32, (4, 1))  # [128, S]
    # sign-folded for the swap32 formulation: -sin on even halves, +sin on odd
    sinx = np.concatenate([-sin32, sin32, -sin32, sin32], axis=0)
    tri = (np.arange(P)[None, :] >= np.arange(P)[:, None]).astype(np.float16)
    idn = np.concatenate([np.eye(64, dtype=np.float16)] * 2, axis=0)  # [128, 64]
    return cosx, sinx, tri, idn


_PERM = np.concatenate([np.arange(0, HD, 2), np.arange(1, HD, 2)])  # deinterleave


def make_in_maps(x, w_qkv, w_o):
    """Build the 8 per-core input maps from full inputs."""
    cosx, sinx, tri, idn = _host_tables()
    in_maps = []
    for c in range(8):
        b, g = c // 4, c % 4
        xTc = np.ascontiguousarray(x[b].T).astype(np.float16)
        cols = []
        for hq in range(NHL * g, NHL * (g + 1)):
            cols.append(hq * HD + _PERM)
        qcols = np.concatenate(cols)
        cols = []
        for kv in range(NKVL * g, NKVL * (g + 1)):
            cols.append(H + kv * HD + _PERM)
        kcols = np.concatenate(cols)
        cols = []
        for kv in range(NKVL * g, NKVL * (g + 1)):
            cols.append(H + NKV * HD + kv * HD + np.arange(HD))
        vcols = np.concatenate(cols)
        wc = np.concatenate(
            [w_qkv[:, qcols], w_qkv[:, kcols], w_qkv[:, vcols]], axis=1
        ).astype(np.float16)
        woc = w_o[FQ * g:FQ * (g + 1), :].astype(np.float16)
        in_maps.append(
            {
                "xT": xTc,
                "wqkv": wc,
                "wo": woc,
                "cosx": cosx,
                "sinx": sinx,
                "tri": tri,
                "idn": idn,
                "ones": np.ones((P, 1), dtype=np.float16),
            }
        )
    return in_maps


_NC = None


def get_nc():
    global _NC
    if _NC is None:
        _NC = build_bass()
    return _NC


def kernel(x, mask, w_qkv, w_o):
    x = np.asarray(x)
    w_qkv = np.asarray(w_qkv)
    w_o = np.asarray(w_o)
    nc = get_nc()
    in_maps = make_in_maps(x, w_qkv, w_o)
    res = run_bass_kernel_spmd(nc, in_maps, core_ids=list(range(8)))
    out = np.zeros((2, S, H), dtype=np.float32)
    for c in range(8):
        out[c // 4] += res.results[c]["out"]
    return out
